# revision 4
# baseline (speedup 1.0000x reference)
"""Trainium2 Bass kernel for 12-head causal MHA (B=2, S=2048, D=768), fp32 in/out.

Sharding: 8 cores = (batch b in {0,1}) x (head-group hg in {0..3}, 3 heads each).
All matmul operands are fp16 (1 cycle/row on PE, half the DMA bytes); PSUM
accumulation is fp32, so end-to-end rel err is ~7e-4 (budget 2e-2).

Math notes (exact, not approximations):
  - bk drops out of softmax entirely: (q+bq)·(k+bk) = q·k + bq·k + const(row).
  - bv folds to the end: attn rows sum to 1, so attn@(v+bv) = attn@v + bv;
    host adds bv@wo.T + bo once.
  - Only bq is applied on device (per-partition scalar add on the q rows).

Per-core layout/dataflow (block j = 512 q columns, key tile t = 128 keys):
  qkT  [384, S]   packed ([q0;q1] | [k0;k1] | [q2;k2]) in 3 m-groups so each
                  head's (q,k) share a base partition; k2 is re-based to
                  partitions 0..63 by a small SBUF->SBUF DMA (Pool queue).
  xT   [128, NT, KC, 128] tile-major (host pre-arranged): every xT DMA line
                  is one contiguous >=1.5KB run, and the first key tile
                  arrives ~0.6us earlier at kernel start.
  v65  [keys, 65] per (tile, head): v columns + a ones column (softmax denom).
  scores: psum [128 keys, 3 heads, <=512 q] (3 banks), one matmul per head,
          causally trimmed; ONE exp for all 3 heads (Act engine), then the
          diagonal 128-col window is masked in place (Pool, SBUF-only op).
  attnV: out [128 q-subtile, 3*65] in ONE psum bank, chains accumulated over
          persistent et tiles (rhs=v65 moving, ap=65 -> 4x fewer PE cycles
          than the [65, q] layout); chain split into prefix + 2-tile suffix
          so finish work overlaps the Act-paced diagonal groups.
  normalize: denominator is a COLUMN -> per-partition reciprocal + scalar mul
          (no partition broadcast needed).
  transpose: blocks 0-2 use one xbar DMA transpose [q,256]->outT (frees PE
          cycles + both DVE copies); the last block keeps the low-latency
          PE transpose since ypq consumes outT immediately.
  yproj: yT[768, S] partial per head-group, host sums 4 partials per batch;
          the last block's yproj runs per q-subtile inside the tail.

Engines: PE matmuls; Act exp (the attention pacer) + tail copies; DVE psum
copies/bias/normalize; Pool masks + k2x DMAs (GPSIMD cannot access PSUM).
PSUM: scores get a dedicated 2x3-bank slot set, everything else (<=2KB)
shares a 2x1-bank pool; start=True lazily zeroes a whole 2KB bank, so
multi-chain banks start exactly once and accumulate onto pending-zero bytes.
Pipeline: projection chains of later blocks and yproj of earlier blocks are
interleaved into attend(j)'s key-tile groups, weighted so each block stays
PE-bound against its linearly-growing exp load.
"""

import math
from contextlib import ExitStack

import numpy as np

import concourse.bacc as bacc
import concourse.bass as bass
import concourse.mybir as mybir
import concourse.tile as tile

FP32 = mybir.dt.float32
FP16 = mybir.dt.float16

B = 2
S = 2048
D = 768
NH = 12
DK = 64
NCORES = 8
HG = 3  # heads per core
HD = HG * DK  # 192
KC = D // 128  # 6 contraction chunks
SB = 512  # q block
NJ = S // SB  # 4
NT = S // 128  # 16 key tiles
SCALE = 1.0 / math.sqrt(DK)
EXP = mybir.ActivationFunctionType.Exp

# head slices inside the packed qkT layout. Matmul operands must share a base
# partition, so groups are m0=[q0;q1], m1=[k0;k1], m2=[q2;k2]; k2 is DMA-moved
# to partitions 0..63 of a scratch tile so h2's (q2,k2) pair is base-aligned.
_Q_SL = {0: (0, 0), 1: (0, 64), 2: (2, 0)}
_K_SL = {0: (1, 0), 1: (1, 64)}


def build_nc(causal: bool):
    nc = bacc.Bacc(trn_type="TRN2", target_bir_lowering=False, debug=False)

    xT_d = nc.declare_dram_parameter("xT", [128, NT * KC * 128], FP16, isOutput=False)
    # weights arrive pre-rearranged to partition-major [128, KC*cols] on the
    # host so each DMA line is one long contiguous run (full DMA speed)
    wqk_d = nc.declare_dram_parameter("wqkP", [128, KC * 2 * HD], FP16, isOutput=False)
    wv_d = nc.declare_dram_parameter("wvP", [128, KC * HD], FP16, isOutput=False)
    woT_d = nc.declare_dram_parameter("woT", [HD, D], FP16, isOutput=False)
    bq0_d = nc.declare_dram_parameter("bq0", [128], FP32, isOutput=False)
    bq1_d = nc.declare_dram_parameter("bq1", [128], FP32, isOutput=False)
    cm_d = nc.declare_dram_parameter("cmask", [128, HG * 128], FP16, isOutput=False)
    id_d = nc.declare_dram_parameter("ident", [128, 128], FP16, isOutput=False)
    yT_d = nc.declare_dram_parameter("yT", [D, S], FP16, isOutput=True)

    with tile.TileContext(nc) as tc, ExitStack() as ctx:
        consts = ctx.enter_context(tc.tile_pool(name="consts", bufs=1))

        xT_sb = consts.tile([128, NT, KC, 128], FP16)
        wqk_sb = consts.tile([128, KC, 2 * HD], FP16)  # packed q|k m-groups
        wv_sb = consts.tile([128, KC, HD], FP16)
        woT0_sb = consts.tile([128, D], FP16)
        woT1_sb = consts.tile([64, D], FP16)
        bq0_sb = consts.tile([128, 1], FP32)
        bq1_sb = consts.tile([128, 1], FP32)
        cm_sb = consts.tile([128, HG, 128], FP16)  # triangle keep-mask x3 heads
        id_sb = consts.tile([128, 128], FP16)
        qkT_sb = consts.tile([128, 3, S], FP16)
        k2x_sb = consts.tile([64, S], FP16)  # k2 re-based to partitions 0..63
        v65_sb = consts.tile([128, NT, HG, 65], FP16)
        # outT[:, 0, s] = head-dims 0..127; outT[0:64, 1, s] = dims 128..191
        # (rows 64:128 of half 1 are junk from the padded transpose)
        outT_sb = consts.tile([128, 2, S], FP16)

        # ---- input DMAs, all on the SP queue. Order matters: v-weights and
        # x block 0 land first so the first vproj chain starts ~2us in.
        xT_r = xT_d.ap().rearrange("p (t c w) -> p t c w", t=NT, c=KC)

        def load_x_block(jb):
            nc.sync.dma_start(
                out=xT_sb[:, 4 * jb : 4 * jb + 4],
                in_=xT_r[:, 4 * jb : 4 * jb + 4],
            )

        # first transfers fan out across SP/DVE/Act queues so their fixed
        # issue latencies (seq+hwdge+dge ~1.8us each) overlap at kernel start
        nc.sync.dma_start(
            out=wv_sb, in_=wv_d.ap().rearrange("p (c n) -> p c n", c=KC)
        )
        nc.gpsimd.dma_start(out=xT_sb[:, 0:1], in_=xT_r[:, 0:1])
        nc.scalar.dma_start(out=xT_sb[:, 1:4], in_=xT_r[:, 1:4])
        nc.sync.dma_start(
            out=wqk_sb, in_=wqk_d.ap().rearrange("p (c n) -> p c n", c=KC)
        )
        nc.sync.dma_start(out=bq0_sb, in_=bq0_d.ap().rearrange("(p o) -> p o", o=1))
        nc.sync.dma_start(out=bq1_sb, in_=bq1_d.ap().rearrange("(p o) -> p o", o=1))
        load_x_block(1)
        nc.sync.dma_start(out=cm_sb, in_=cm_d.ap().rearrange("p (h c) -> p h c", h=HG))
        nc.sync.dma_start(out=id_sb, in_=id_d.ap())
        nc.sync.dma_start(out=woT0_sb, in_=woT_d.ap()[0:128, :])
        nc.sync.dma_start(out=woT1_sb, in_=woT_d.ap()[128:HD, :])
        load_x_block(2)
        load_x_block(3)

        nc.vector.memset(v65_sb[:, :, :, 64:65], 1.0)  # softmax-denominator col
        # ---- PSUM pools: scores get a dedicated 2x3-bank slot set so the
        # next scores group never waits on exp draining a filler's slot; all
        # other psum tiles are <=2KB and share a 2x1-bank small pool.
        sc_pool = ctx.enter_context(tc.tile_pool(name="sc", bufs=2, space="PSUM"))
        fl_pool = ctx.enter_context(tc.tile_pool(name="fl", bufs=2, space="PSUM"))
        et_pool = ctx.enter_context(tc.tile_pool(name="et", bufs=18))
        nrm_pool = ctx.enter_context(tc.tile_pool(name="nrm", bufs=3))
        rc_pool = ctx.enter_context(tc.tile_pool(name="rc", bufs=3))
        yt_pool = ctx.enter_context(tc.tile_pool(name="yt", bufs=3))

        def vp_unit(t):
            """Project v for key tile t into v65 (natural layout, ap=192)."""

            def emit():
                vpu = fl_pool.tile([128, 256], FP32, name="vpu", tag="f")
                for c in range(KC):
                    nc.tensor.matmul(
                        vpu[:, 0:HD],
                        lhsT=xT_sb[:, t, c, :],
                        rhs=wv_sb[:, c, :],
                        start=(c == 0),
                        stop=(c == KC - 1),
                    )
                # one strided copy; GPSIMD cannot touch PSUM, so DVE
                nc.vector.tensor_copy(
                    v65_sb[:, t, :, 0:64],
                    vpu[:, 0:HD].rearrange("p (h d) -> p h d", h=HG),
                )

            return emit

        def qk_unit(j, m):
            """Project packed qk m-group for q block j (transposed, ap=512)."""

            def emit():
                qku = fl_pool.tile([128, SB], FP32, name="qku", tag="f")
                for c in range(KC):
                    nc.tensor.matmul(
                        qku,
                        lhsT=wqk_sb[:, c, m * 128 : (m + 1) * 128],
                        rhs=xT_sb[:, 4 * j : 4 * j + 4, c, :],
                        start=(c == 0),
                        stop=(c == KC - 1),
                    )
                jsl = slice(j * SB, (j + 1) * SB)
                dst = qkT_sb[:, m, jsl]
                if m == 0:
                    nc.vector.tensor_scalar_add(dst, qku, bq0_sb)
                elif m == 2:
                    nc.vector.tensor_scalar_add(dst, qku, bq1_sb)
                    # Pool (swdge) queue: the SP queue is busy streaming bulk
                    # inputs and would delay this small hop by several us
                    nc.gpsimd.dma_start(
                        out=k2x_sb[0:64, jsl], in_=qkT_sb[64:128, 2, jsl]
                    )
                else:
                    nc.vector.tensor_copy(dst, qku)

            return emit

        def yp_unit(j, dt, ystate, on_act=False):
            """Output projection for one 128-row chunk of yT, q block j.

            on_act: psum->sbuf copy on the Act engine (fills Act's idle
            window at block boundaries; no PE-downstream dependency)."""

            def emit():
                ypu = fl_pool.tile([128, SB], FP32, name="ypu", tag="f")
                jsl = slice(j * SB, (j + 1) * SB)
                nc.tensor.matmul(
                    ypu,
                    lhsT=woT0_sb[:, dt * 128 : (dt + 1) * 128],
                    rhs=outT_sb[:, 0, jsl],
                    start=True,
                    stop=False,
                )
                nc.tensor.matmul(
                    ypu,
                    lhsT=woT1_sb[0:64, dt * 128 : (dt + 1) * 128],
                    rhs=outT_sb[0:64, 1, jsl],
                    start=False,
                    stop=True,
                )
                sub = dt % 3
                if sub == 0:
                    ystate["yt"] = yt_pool.tile([128, 3, SB], FP16, name="yt")
                (nc.scalar.copy if on_act else nc.vector.tensor_copy)(
                    ystate["yt"][:, sub, :], ypu
                )
                if sub == 2:
                    nc.sync.dma_start(
                        out=yT_d.ap().rearrange("(g p) s -> p g s", p=128)[
                            :, dt - 2 : dt + 1, jsl
                        ],
                        in_=ystate["yt"],
                    )

            return emit

        def av_chain(j, qs, ets, t_lo, t_hi, avst):
            """attnV accumulation for q-subtile qs of block j over key tiles
            t_lo..t_hi. av psum unit is [128, 3*65] in ONE bank: only the
            first write may use start=True (it lazily zeroes the whole bank);
            later chains accumulate onto pending-zero bytes."""
            if t_lo == 0:
                avst[qs] = fl_pool.tile([128, 256], FP32, name="av", tag="f")
            av = avst[qs]
            for t in range(t_lo, t_hi + 1):
                for h in range(HG):
                    nc.tensor.matmul(
                        av[:, 65 * h : 65 * h + 65],
                        lhsT=ets[t][:, h, qs * 128 : (qs + 1) * 128],
                        rhs=v65_sb[:, t, h, :],
                        start=(t == 0 and h == 0),
                        stop=(t == t_hi),
                        skip_group_check=True,
                    )

        def finish_qs(j, qs, avst, on_act=False):
            """Normalize q-subtile qs of block j and transpose into outT.

            on_act is unused (kept for experiment parity)."""
            av = avst[qs]
            rc = rc_pool.tile([128, HG], FP32, name="rc")
            nc.vector.reciprocal(rc, av[:, 64 : 65 * HG : 65])
            nrm = nrm_pool.tile([128, 256], FP16, name="nrm")
            nc.gpsimd.memset(nrm[:, HD:256], 0.0)
            for h in range(HG):
                nc.vector.tensor_scalar_mul(
                    nrm[:, h * 64 : (h + 1) * 64],
                    av[:, 65 * h : 65 * h + 64],
                    rc[:, h : h + 1],
                )
            col = slice(j * SB + qs * 128, j * SB + (qs + 1) * 128)
            if j < NJ - 1:
                # single xbar DMA transpose (SP queue): [128, 256] -> both
                # outT halves at once. Frees the PE transposes and both DVE
                # psum->sbuf copies; outT is consumed 1-2 blocks later so the
                # dge latency is invisible.
                nc.sync.dma_start_transpose(outT_sb[:, :, col], nrm)
            else:
                # last block: outT feeds ypq immediately; keep the low-latency
                # PE transpose. both land in one psum bank: the second must
                # not start (start lazily zeroes the bank, wiping the first)
                tp = fl_pool.tile([128, 256], FP16, name="tp", tag="f")
                nc.tensor.transpose(tp[0:128, 0:128], nrm[:, 0:128], id_sb)
                nc.tensor.matmul(
                    tp[0:64, 128:256],
                    lhsT=nrm[:, 128:HD],
                    rhs=id_sb,
                    is_transpose=True,
                    start=False,
                    stop=True,
                    skip_group_check=True,
                )
                nc.vector.tensor_copy(outT_sb[:, 0, col], tp[0:128, 0:128])
                nc.vector.tensor_copy(outT_sb[0:64, 1, col], tp[0:64, 128:256])
            if causal and j == NJ - 1:
                # last block: project this q-subtile's columns immediately so
                # the output projection+DMA pipelines into the attention tail
                ytq = yt_pool.tile([128, KC, 128], FP16, name="ytq", tag="ytq")
                for half in range(2):
                    ypq = fl_pool.tile([128, 3, 128], FP32, name="ypq", tag="f")
                    for sub in range(3):
                        dt = 3 * half + sub
                        nc.tensor.matmul(
                            ypq[:, sub, :],
                            lhsT=woT0_sb[:, dt * 128 : (dt + 1) * 128],
                            rhs=outT_sb[:, 0, col],
                            start=(sub == 0),
                            stop=False,
                            skip_group_check=True,
                        )
                        nc.tensor.matmul(
                            ypq[:, sub, :],
                            lhsT=woT1_sb[0:64, dt * 128 : (dt + 1) * 128],
                            rhs=outT_sb[0:64, 1, col],
                            start=False,
                            stop=(sub == 2),
                            skip_group_check=True,
                        )
                    # Act: it is idle in the kernel tail where these run,
                    # while DVE is the serial bottleneck of the final rounds.
                    # For the very last q-subtile, the second half goes via
                    # DVE so the two final copy->DMA chains run in parallel.
                    if half == 1 and qs == 3:
                        nc.vector.tensor_copy(ytq[:, 3:6, :], ypq)
                    else:
                        nc.scalar.copy(ytq[:, 3 * half : 3 * half + 3, :], ypq)
                    # per-half DMA: the first half flies while the second is
                    # still copying, halving the final DMA on the tail path
                    nc.sync.dma_start(
                        out=yT_d.ap().rearrange("(g p) s -> p g s", p=128)[
                            :, 3 * half : 3 * half + 3, col
                        ],
                        in_=ytq[:, 3 * half : 3 * half + 3, :],
                    )

        ets_all = {}

        def attend_block(j, units, prefetch_next=True):
            """Attention for q block j; pops filler units between key tiles.

            attnV chains run per q-subtile over persistent et tiles, one
            group after the diagonal tile's exp, so the PE never waits on
            the Act engine in program order. The next block's first scores
            group is emitted inside this block's last iteration: its psum
            half is free by then, and its exp fills Act's boundary bubble.
            """
            tend = 4 * (j + 1) if causal else NT
            ets = ets_all.setdefault(j, {})
            avst = {}
            nunits = len(units)
            upos = 0

            def scores_group(t, jj=j):
                d = t - 4 * jj if causal else -1  # diag index within block
                off = 128 * d if (causal and d >= 0) else 0
                sp = sc_pool.tile([128, HG, SB], FP32, name="sp", tag="s")
                for h in range(HG):
                    if h < 2:
                        kg, kp = _K_SL[h]
                        k_ap = qkT_sb[kp : kp + 64, kg, t * 128 : (t + 1) * 128]
                    else:
                        k_ap = k2x_sb[0:64, t * 128 : (t + 1) * 128]
                    qg, qp = _Q_SL[h]
                    nc.tensor.matmul(
                        sp[:, h, off:],
                        lhsT=k_ap,
                        rhs=qkT_sb[qp : qp + 64, qg, jj * SB + off : (jj + 1) * SB],
                        start=True,
                        stop=True,
                    )
                et = et_pool.tile([128, HG, SB], FP16, name="et")
                nc.scalar.activation(et[:, :, off:], sp[:, :, off:], EXP, scale=SCALE)
                if causal and d >= 0:
                    # intra-tile causal triangle: keep key p iff p <= local q.
                    # Pool engine: SBUF-only op, and Pool is otherwise idle
                    nc.gpsimd.tensor_mul(
                        et[:, :, off : off + 128], et[:, :, off : off + 128], cm_sb
                    )
                ets_all.setdefault(jj, {})[t] = et

            # Each diag subtile qs's chain is split: a prefix over key tiles
            # 0..4j+qs-2 (emitted at group 4j+qs, needs only exp(t-2)) and a
            # 2-tile suffix + normalize at group 4j+qs+1, so at most two av
            # tiles are alive (2 small-pool slots) and the block tail is tiny.
            for t in range(tend):
                if t not in ets:
                    scores_group(t)
                # front-load the first 4 units (this block's v-projections
                # must land a few groups before their key tiles are consumed);
                # drain all units 2 groups before block end so the next
                # block's qk projections (and their k2x hop) are ready
                uend = max(
                    min(t + 1, 4, nunits),
                    min(nunits, (t + 1) * nunits // max(1, tend - 2)),
                )
                while upos < uend:
                    units[upos]()
                    upos += 1
                d = t - 4 * j if causal else -1
                if causal and 0 <= d:
                    if 4 * j + d - 2 >= 0:
                        av_chain(j, d, ets, 0, 4 * j + d - 2, avst)
                    if d >= 1:
                        qs = d - 1
                        av_chain(j, qs, ets, max(0, 4 * j + qs - 1), 4 * j + qs, avst)
                        finish_qs(j, qs, avst)
            if causal:
                av_chain(j, 3, ets, max(0, 4 * j + 2), 4 * j + 3, avst)
                if prefetch_next and j < NJ - 1:
                    scores_group(0, jj=j + 1)
                finish_qs(j, 3, avst, on_act=(j < NJ - 1))
            else:
                for qs in range(4):
                    av_chain(j, qs, ets, 0, NT - 1, avst)
                    finish_qs(j, qs, avst)
            while upos < nunits:
                units[upos]()
                upos += 1

        if causal:
            # Filler placement balances PE work against each block's Act
            # (exp) load, which grows ~linearly with j: v-projections are
            # deferred into their OWN block (legal: tile 4j+st is first read
            # two groups later), yproj of finished blocks goes to the late,
            # Act-heavy blocks.
            for u in [vp_unit(t) for t in range(4)] + [qk_unit(0, m) for m in (2, 0, 1)]:
                u()
            ys = [{} for _ in range(NJ)]
            # qk(j+1) leads each unit list: the next block's first scores
            # gate on it, and the front-load rule pops one unit per group
            # m2 first within each qk triple: it triggers the k2x re-basing
            # DMA whose latency otherwise stalls the next block's first scores
            units_by_block = {
                0: [qk_unit(1, m) for m in (2, 0, 1)],
                1: [qk_unit(2, 2)]
                + [vp_unit(4 + st) for st in range(4)]
                + [qk_unit(2, 0), qk_unit(2, 1)],
                2: [qk_unit(3, 2)]
                + [vp_unit(8 + st) for st in range(4)]
                + [qk_unit(3, 0), qk_unit(3, 1)]
                + [yp_unit(0, dt, ys[0]) for dt in range(KC)],
                3: [vp_unit(12 + st) for st in range(4)]
                + [yp_unit(1, dt, ys[1]) for dt in range(KC)]
                + [yp_unit(2, dt, ys[2]) for dt in range(KC)],
            }
            for j in range(NJ):
                attend_block(j, units_by_block[j])
            # (last block's output projection is emitted per q-subtile inside
            # finish_qs so it pipelines into the attention tail)
        else:
            for t in range(NT):
                vp_unit(t)()
            for j in range(NJ):
                for m in range(3):
                    qk_unit(j, m)()
            for j in range(NJ):
                attend_block(j, [])
                ystate = {}
                for dt in range(KC):
                    yp_unit(j, dt, ystate)()

    nc.finalize()
    return nc


_NC_CACHE: dict[bool, object] = {}


def get_nc(causal: bool):
    if causal not in _NC_CACHE:
        _NC_CACHE[causal] = build_nc(causal)
    return _NC_CACHE[causal]


def make_in_maps(x, wq, bq, wk, bk, wv, bv, wo, bo):
    """Shard full inputs into 8 per-core input maps (all-fp16 operands)."""
    f16 = np.float16
    tri = (np.arange(128)[:, None] <= np.arange(128)[None, :]).astype(f16)
    cm = np.ascontiguousarray(np.tile(tri, (1, HG)))
    ident = np.ascontiguousarray(np.eye(128, dtype=f16))
    x = np.asarray(x, np.float32)
    in_maps = []
    for core in range(NCORES):
        b, hg = divmod(core, NH // HG)
        hs = slice(hg * HD, (hg + 1) * HD)
        qc, kc, vc = (np.asarray(w, np.float32)[hs, :].T for w in (wq, wk, wv))
        wqkT = np.concatenate(
            [qc[:, 0:128], kc[:, 0:128], qc[:, 128:HD], kc[:, 128:HD]], axis=1
        ).astype(f16)
        # pre-rearrange to partition-major [128, KC*cols] so each DMA line is
        # one contiguous run
        wqkP = wqkT.reshape(KC, 128, 2 * HD).transpose(1, 0, 2).reshape(128, -1)
        wvP = vc.astype(f16).reshape(KC, 128, HD).transpose(1, 0, 2).reshape(128, -1)
        bqh = np.asarray(bq, np.float32)[hs]
        bq1p = np.concatenate([bqh[128:HD], np.zeros(64, np.float32)])
        in_maps.append(
            {
                "xT": np.ascontiguousarray(
                    x[b].T.reshape(KC, 128, NT, 128).transpose(1, 2, 0, 3)
                ).astype(f16).reshape(128, -1),
                "wqkP": np.ascontiguousarray(wqkP),
                "wvP": np.ascontiguousarray(wvP),
                "woT": np.ascontiguousarray(np.asarray(wo, np.float32)[:, hs].T).astype(f16),
                "bq0": np.ascontiguousarray(bqh[0:128]),
                "bq1": np.ascontiguousarray(bq1p),
                "cmask": cm,
                "ident": ident,
            }
        )
    return in_maps


def combine_outputs(results, wo, bv, bo):
    """Sum head-group partials per batch, transpose, add bv@wo.T + bo."""
    y = np.empty((B, S, D), np.float32)
    ng = NH // HG
    corr = (
        np.asarray(bv, np.float64) @ np.asarray(wo, np.float64).T
        + np.asarray(bo, np.float64)
    ).astype(np.float32)
    for b in range(B):
        acc = results[b * ng]["yT"].astype(np.float32)
        for g in range(1, ng):
            acc = acc + results[b * ng + g]["yT"].astype(np.float32)
        y[b] = acc.T + corr[None, :]
    return y


def kernel(x, wq, bq, wk, bk, wv, bv, wo, bo, mask, _trace=False):
    from concourse.bass_utils import run_bass_kernel_spmd

    causal = bool(np.asarray(mask).item())
    nc = get_nc(causal)
    in_maps = make_in_maps(x, wq, bq, wk, bk, wv, bv, wo, bo)
    res = run_bass_kernel_spmd(nc, in_maps, list(range(NCORES)), trace=_trace)
    y = combine_outputs(res.results, wo, bv, bo)
    if _trace:
        return y, res
    return y



# revision 6
# speedup vs baseline: 1.0150x; 1.0150x over previous
"""Trainium2 Bass kernel for 12-head causal MHA (B=2, S=2048, D=768), fp32 in/out.

Sharding: 8 cores = (batch b in {0,1}) x (head-group hg in {0..3}, 3 heads each).
All matmul operands are fp16 (1 cycle/row on PE, half the DMA bytes); PSUM
accumulation is fp32, so end-to-end rel err is ~7e-4 (budget 2e-2).

Math notes (exact, not approximations):
  - bk drops out of softmax entirely: (q+bq)·(k+bk) = q·k + bq·k + const(row).
  - bv folds to the end: attn rows sum to 1, so attn@(v+bv) = attn@v + bv;
    host adds bv@wo.T + bo once.
  - Only bq is applied on device (per-partition scalar add on the q rows).

Per-core layout/dataflow (block j = 512 q columns, key tile t = 128 keys):
  qkT  [384, S]   packed ([q0;q1] | [k0;k1] | [q2;k2]) in 3 m-groups so each
                  head's (q,k) share a base partition; k2 is re-based to
                  partitions 0..63 by a small SBUF->SBUF DMA (Pool queue).
  xT   [128, NT, KC, 128] tile-major (host pre-arranged): every xT DMA line
                  is one contiguous >=1.5KB run, and the first key tile
                  arrives ~0.6us earlier at kernel start.
  v65  [keys, 65] per (tile, head): v columns + a ones column (softmax denom).
  scores: psum [128 keys, 3 heads, <=512 q] (3 banks), one matmul per head,
          causally trimmed; ONE exp for all 3 heads (Act engine), then the
          diagonal 128-col window is masked in place (Pool, SBUF-only op).
  attnV: out [128 q-subtile, 3*65] in ONE psum bank, chains accumulated over
          persistent et tiles (rhs=v65 moving, ap=65 -> 4x fewer PE cycles
          than the [65, q] layout); chain split into prefix + 2-tile suffix
          so finish work overlaps the Act-paced diagonal groups.
  normalize: denominator is a COLUMN -> per-partition reciprocal + scalar mul
          (no partition broadcast needed).
  transpose: blocks 0-2 use one xbar DMA transpose [q,256]->outT (frees PE
          cycles + both DVE copies); the last block keeps the low-latency
          PE transpose since ypq consumes outT immediately.
  yproj: yT[768, S] partial per head-group, host sums 4 partials per batch;
          the last block's yproj runs per q-subtile inside the tail.

Engines: PE matmuls; Act exp (the attention pacer) + tail copies; DVE psum
copies/bias/normalize; Pool masks + k2x DMAs (GPSIMD cannot access PSUM).
PSUM: scores get a dedicated 2x3-bank slot set, everything else (<=2KB)
shares a 2x1-bank pool; start=True lazily zeroes a whole 2KB bank, so
multi-chain banks start exactly once and accumulate onto pending-zero bytes.
Pipeline: projection chains of later blocks and yproj of earlier blocks are
interleaved into attend(j)'s key-tile groups, weighted so each block stays
PE-bound against its linearly-growing exp load.
"""

import math
from contextlib import ExitStack

import numpy as np

import concourse.bacc as bacc
import concourse.bass as bass
import concourse.mybir as mybir
import concourse.tile as tile

FP32 = mybir.dt.float32
FP16 = mybir.dt.float16

B = 2
S = 2048
D = 768
NH = 12
DK = 64
NCORES = 8
HG = 3  # heads per core
HD = HG * DK  # 192
KC = D // 128  # 6 contraction chunks
SB = 512  # q block
NJ = S // SB  # 4
NT = S // 128  # 16 key tiles
SCALE = 1.0 / math.sqrt(DK)
EXP = mybir.ActivationFunctionType.Exp

# head slices inside the packed qkT layout. Matmul operands must share a base
# partition, so groups are m0=[q0;q1], m1=[k0;k1], m2=[q2;k2]; k2 is DMA-moved
# to partitions 0..63 of a scratch tile so h2's (q2,k2) pair is base-aligned.
_Q_SL = {0: (0, 0), 1: (0, 64), 2: (2, 0)}
_K_SL = {0: (1, 0), 1: (1, 64)}


def build_nc(causal: bool):
    nc = bacc.Bacc(trn_type="TRN2", target_bir_lowering=False, debug=False)

    xT_d = nc.declare_dram_parameter("xT", [128, NT * KC * 128], FP16, isOutput=False)
    # weights arrive pre-rearranged to partition-major [128, KC*cols] on the
    # host so each DMA line is one long contiguous run (full DMA speed)
    wqk_d = nc.declare_dram_parameter("wqkP", [128, KC * 2 * HD], FP16, isOutput=False)
    wv_d = nc.declare_dram_parameter("wvP", [128, KC * HD], FP16, isOutput=False)
    woT_d = nc.declare_dram_parameter("woT", [HD, D], FP16, isOutput=False)
    bq0_d = nc.declare_dram_parameter("bq0", [128], FP32, isOutput=False)
    bq1_d = nc.declare_dram_parameter("bq1", [128], FP32, isOutput=False)
    cm_d = nc.declare_dram_parameter("cmask", [128, HG * 128], FP16, isOutput=False)
    id_d = nc.declare_dram_parameter("ident", [128, 128], FP16, isOutput=False)
    yT_d = nc.declare_dram_parameter("yT", [D, S], FP16, isOutput=True)
    # block-3 columns land tile-major here: contiguous 768B runs per
    # partition-half -> mult-1 DMA speed in the latency-critical kernel tail
    yTq_d = nc.declare_dram_parameter("yTq", [128, 4 * KC * 128], FP16, isOutput=True)

    with tile.TileContext(nc) as tc, ExitStack() as ctx:
        consts = ctx.enter_context(tc.tile_pool(name="consts", bufs=1))

        xT_sb = consts.tile([128, NT, KC, 128], FP16)
        wqk_sb = consts.tile([128, KC, 2 * HD], FP16)  # packed q|k m-groups
        wv_sb = consts.tile([128, KC, HD], FP16)
        woT0_sb = consts.tile([128, D], FP16)
        woT1_sb = consts.tile([64, D], FP16)
        bq0_sb = consts.tile([128, 1], FP32)
        bq1_sb = consts.tile([128, 1], FP32)
        cm_sb = consts.tile([128, HG, 128], FP16)  # triangle keep-mask x3 heads
        id_sb = consts.tile([128, 128], FP16)
        qkT_sb = consts.tile([128, 3, S], FP16)
        k2x_sb = consts.tile([64, S], FP16)  # k2 re-based to partitions 0..63
        v65_sb = consts.tile([128, NT, HG, 65], FP16)
        # outT[:, 0, s] = head-dims 0..127; outT[0:64, 1, s] = dims 128..191
        # (rows 64:128 of half 1 are junk from the padded transpose)
        outT_sb = consts.tile([128, 2, S], FP16)

        # ---- input DMAs, all on the SP queue. Order matters: v-weights and
        # x block 0 land first so the first vproj chain starts ~2us in.
        xT_r = xT_d.ap().rearrange("p (t c w) -> p t c w", t=NT, c=KC)

        def load_x_block(jb):
            nc.sync.dma_start(
                out=xT_sb[:, 4 * jb : 4 * jb + 4],
                in_=xT_r[:, 4 * jb : 4 * jb + 4],
            )

        # first transfers fan out across SP/DVE/Act queues so their fixed
        # issue latencies (seq+hwdge+dge ~1.8us each) overlap at kernel start
        nc.sync.dma_start(
            out=wv_sb, in_=wv_d.ap().rearrange("p (c n) -> p c n", c=KC)
        )
        nc.gpsimd.dma_start(out=xT_sb[:, 0:1], in_=xT_r[:, 0:1])
        nc.scalar.dma_start(out=xT_sb[:, 1:4], in_=xT_r[:, 1:4])
        nc.sync.dma_start(
            out=wqk_sb, in_=wqk_d.ap().rearrange("p (c n) -> p c n", c=KC)
        )
        nc.sync.dma_start(out=bq0_sb, in_=bq0_d.ap().rearrange("(p o) -> p o", o=1))
        nc.sync.dma_start(out=bq1_sb, in_=bq1_d.ap().rearrange("(p o) -> p o", o=1))
        load_x_block(1)
        nc.sync.dma_start(out=cm_sb, in_=cm_d.ap().rearrange("p (h c) -> p h c", h=HG))
        nc.sync.dma_start(out=id_sb, in_=id_d.ap())
        nc.sync.dma_start(out=woT0_sb, in_=woT_d.ap()[0:128, :])
        nc.sync.dma_start(out=woT1_sb, in_=woT_d.ap()[128:HD, :])
        load_x_block(2)
        load_x_block(3)

        nc.vector.memset(v65_sb[:, :, :, 64:65], 1.0)  # softmax-denominator col
        # ---- PSUM pools: scores get a dedicated 2x3-bank slot set so the
        # next scores group never waits on exp draining a filler's slot; all
        # other psum tiles are <=2KB and share a 2x1-bank small pool.
        sc_pool = ctx.enter_context(tc.tile_pool(name="sc", bufs=2, space="PSUM"))
        fl_pool = ctx.enter_context(tc.tile_pool(name="fl", bufs=2, space="PSUM"))
        et_pool = ctx.enter_context(tc.tile_pool(name="et", bufs=18))
        nrm_pool = ctx.enter_context(tc.tile_pool(name="nrm", bufs=3))
        rc_pool = ctx.enter_context(tc.tile_pool(name="rc", bufs=3))
        yt_pool = ctx.enter_context(tc.tile_pool(name="yt", bufs=3))

        def vp_unit(t):
            """Project v for key tile t into v65 (natural layout, ap=192)."""

            def emit():
                vpu = fl_pool.tile([128, 256], FP32, name="vpu", tag="f")
                for c in range(KC):
                    nc.tensor.matmul(
                        vpu[:, 0:HD],
                        lhsT=xT_sb[:, t, c, :],
                        rhs=wv_sb[:, c, :],
                        start=(c == 0),
                        stop=(c == KC - 1),
                    )
                # one strided copy; GPSIMD cannot touch PSUM, so DVE
                nc.vector.tensor_copy(
                    v65_sb[:, t, :, 0:64],
                    vpu[:, 0:HD].rearrange("p (h d) -> p h d", h=HG),
                )

            return emit

        def qk_units(j, m):
            """Project packed qk m-group for q block j (transposed, ap=512),
            split into two filler units of 3 contraction chunks each so the
            scheduler can pace them at sub-group granularity."""
            state = {}

            def piece(c0, c1):
                def emit():
                    if c0 == 0:
                        state["qku"] = fl_pool.tile(
                            [128, SB], FP32, name="qku", tag="f"
                        )
                    for c in range(c0, c1):
                        nc.tensor.matmul(
                            state["qku"],
                            lhsT=wqk_sb[:, c, m * 128 : (m + 1) * 128],
                            rhs=xT_sb[:, 4 * j : 4 * j + 4, c, :],
                            start=(c == 0),
                            stop=(c == KC - 1),
                        )
                return emit

            emit_a = piece(0, 2)
            emit_mid = piece(2, 4)

            def emit_b():
                qku = state["qku"]
                for c in range(4, KC):
                    nc.tensor.matmul(
                        qku,
                        lhsT=wqk_sb[:, c, m * 128 : (m + 1) * 128],
                        rhs=xT_sb[:, 4 * j : 4 * j + 4, c, :],
                        start=False,
                        stop=(c == KC - 1),
                    )
                jsl = slice(j * SB, (j + 1) * SB)
                dst = qkT_sb[:, m, jsl]
                if m == 0:
                    nc.vector.tensor_scalar_add(dst, qku, bq0_sb)
                elif m == 2:
                    nc.vector.tensor_scalar_add(dst, qku, bq1_sb)
                    # Pool (swdge) queue: the SP queue is busy streaming bulk
                    # inputs and would delay this small hop by several us
                    nc.gpsimd.dma_start(
                        out=k2x_sb[0:64, jsl], in_=qkT_sb[64:128, 2, jsl]
                    )
                else:
                    nc.vector.tensor_copy(dst, qku)

            return [emit_a, emit_mid, emit_b]

        def yp_units(j, dt, ystate):
            """Output projection for one 128-row chunk of yT, q block j."""

            def emit():
                ypu = fl_pool.tile([128, SB], FP32, name="ypu", tag="f")
                jsl = slice(j * SB, (j + 1) * SB)
                nc.tensor.matmul(
                    ypu,
                    lhsT=woT0_sb[:, dt * 128 : (dt + 1) * 128],
                    rhs=outT_sb[:, 0, jsl],
                    start=True,
                    stop=False,
                )
                nc.tensor.matmul(
                    ypu,
                    lhsT=woT1_sb[0:64, dt * 128 : (dt + 1) * 128],
                    rhs=outT_sb[0:64, 1, jsl],
                    start=False,
                    stop=True,
                )
                sub = dt % 3
                if sub == 0:
                    ystate["yt"] = yt_pool.tile([128, 3, SB], FP16, name="yt")
                nc.vector.tensor_copy(ystate["yt"][:, sub, :], ypu)
                if sub == 2:
                    nc.sync.dma_start(
                        out=yT_d.ap().rearrange("(g p) s -> p g s", p=128)[
                            :, dt - 2 : dt + 1, jsl
                        ],
                        in_=ystate["yt"],
                    )

            return [emit]

        def av_chain(j, qs, ets, t_lo, t_hi, avst):
            """attnV accumulation for q-subtile qs of block j over key tiles
            t_lo..t_hi. av psum unit is [128, 3*65] in ONE bank: only the
            first write may use start=True (it lazily zeroes the whole bank);
            later chains accumulate onto pending-zero bytes."""
            if t_lo == 0:
                avst[qs] = fl_pool.tile([128, 256], FP32, name="av", tag="f")
            av = avst[qs]
            for t in range(t_lo, t_hi + 1):
                for h in range(HG):
                    nc.tensor.matmul(
                        av[:, 65 * h : 65 * h + 65],
                        lhsT=ets[t][:, h, qs * 128 : (qs + 1) * 128],
                        rhs=v65_sb[:, t, h, :],
                        start=(t == 0 and h == 0),
                        stop=(t == t_hi),
                        skip_group_check=True,
                    )

        ypq_defer = []

        def finish_qs(j, qs, avst, on_act=False):
            """Normalize q-subtile qs of block j and transpose into outT.

            on_act is unused (kept for experiment parity)."""
            av = avst[qs]
            rc = rc_pool.tile([128, HG], FP32, name="rc")
            nc.vector.reciprocal(rc, av[:, 64 : 65 * HG : 65])
            nrm = nrm_pool.tile([128, 256], FP16, name="nrm")
            nc.gpsimd.memset(nrm[:, HD:256], 0.0)
            for h in range(HG):
                nc.vector.tensor_scalar_mul(
                    nrm[:, h * 64 : (h + 1) * 64],
                    av[:, 65 * h : 65 * h + 64],
                    rc[:, h : h + 1],
                )
            col = slice(j * SB + qs * 128, j * SB + (qs + 1) * 128)
            if j < NJ - 1:
                # single xbar DMA transpose (SP queue): [128, 256] -> both
                # outT halves at once. Frees the PE transposes and both DVE
                # psum->sbuf copies; outT is consumed 1-2 blocks later so the
                # dge latency is invisible.
                nc.sync.dma_start_transpose(outT_sb[:, :, col], nrm)
            else:
                # last block: outT feeds ypq immediately; keep the low-latency
                # PE transpose. both land in one psum bank: the second must
                # not start (start lazily zeroes the bank, wiping the first)
                tp = fl_pool.tile([128, 256], FP16, name="tp", tag="f")
                nc.tensor.transpose(tp[0:128, 0:128], nrm[:, 0:128], id_sb)
                nc.tensor.matmul(
                    tp[0:64, 128:256],
                    lhsT=nrm[:, 128:HD],
                    rhs=id_sb,
                    is_transpose=True,
                    start=False,
                    stop=True,
                    skip_group_check=True,
                )
                nc.vector.tensor_copy(outT_sb[:, 0, col], tp[0:128, 0:128])
                nc.vector.tensor_copy(outT_sb[0:64, 1, col], tp[0:64, 128:256])
            if causal and j == NJ - 1:
                # last block: project this q-subtile's columns; emission is
                # deferred one group so the PE matmuls overlap the NEXT
                # subtile's DVE normalize/copy phase instead of waiting on
                # this one's.
                def ypq_emit(qs=qs, col=col):
                    _ypq_body(j, qs, col)
                ypq_defer.append(ypq_emit)

        def _ypq_body(j, qs, col):
            if True:
                ytq = yt_pool.tile([128, KC, 128], FP16, name="ytq", tag="ytq")
                for half in range(2):
                    ypq = fl_pool.tile([128, 3, 128], FP32, name="ypq", tag="f")
                    for sub in range(3):
                        dt = 3 * half + sub
                        nc.tensor.matmul(
                            ypq[:, sub, :],
                            lhsT=woT0_sb[:, dt * 128 : (dt + 1) * 128],
                            rhs=outT_sb[:, 0, col],
                            start=(sub == 0),
                            stop=False,
                            skip_group_check=True,
                        )
                        nc.tensor.matmul(
                            ypq[:, sub, :],
                            lhsT=woT1_sb[0:64, dt * 128 : (dt + 1) * 128],
                            rhs=outT_sb[0:64, 1, col],
                            start=False,
                            stop=(sub == 2),
                            skip_group_check=True,
                        )
                    # Act: it is idle in the kernel tail where these run,
                    # while DVE is the serial bottleneck of the final rounds.
                    # For the very last q-subtile, the second half goes via
                    # DVE so the two final copy->DMA chains run in parallel.
                    if half == 1 and qs == 3:
                        nc.vector.tensor_copy(ytq[:, 3:6, :], ypq)
                    else:
                        nc.scalar.copy(ytq[:, 3 * half : 3 * half + 3, :], ypq)
                    # per-half DMA into the tile-major tail tensor: 768B
                    # contiguous runs -> mult-1 DMA speed (the [p g s] yT
                    # layout would give 256B runs at 2x transfer cost)
                    nc.sync.dma_start(
                        out=yTq_d.ap().rearrange(
                            "p (q c w) -> p q c w", q=4, c=KC
                        )[:, qs, 3 * half : 3 * half + 3],
                        in_=ytq[:, 3 * half : 3 * half + 3, :],
                    )

        ets_all = {}

        def attend_block(j, units, prefetch_next=True):
            """Attention for q block j; pops filler units between key tiles.

            attnV chains run per q-subtile over persistent et tiles, one
            group after the diagonal tile's exp, so the PE never waits on
            the Act engine in program order. The next block's first scores
            group is emitted inside this block's last iteration: its psum
            half is free by then, and its exp fills Act's boundary bubble.
            """
            tend = 4 * (j + 1) if causal else NT
            ets = ets_all.setdefault(j, {})
            del ypq_defer[:]
            avst = {}
            nunits = len(units)
            upos = 0

            def scores_group(t, jj=j):
                d = t - 4 * jj if causal else -1  # diag index within block
                off = 128 * d if (causal and d >= 0) else 0
                sp = sc_pool.tile([128, HG, SB], FP32, name="sp", tag="s")
                for h in range(HG):
                    if h < 2:
                        kg, kp = _K_SL[h]
                        k_ap = qkT_sb[kp : kp + 64, kg, t * 128 : (t + 1) * 128]
                    else:
                        k_ap = k2x_sb[0:64, t * 128 : (t + 1) * 128]
                    qg, qp = _Q_SL[h]
                    nc.tensor.matmul(
                        sp[:, h, off:],
                        lhsT=k_ap,
                        rhs=qkT_sb[qp : qp + 64, qg, jj * SB + off : (jj + 1) * SB],
                        start=True,
                        stop=True,
                    )
                et = et_pool.tile([128, HG, SB], FP16, name="et")
                nc.scalar.activation(et[:, :, off:], sp[:, :, off:], EXP, scale=SCALE)
                if causal and d >= 0:
                    # intra-tile causal triangle: keep key p iff p <= local q.
                    # Pool engine: SBUF-only op, and Pool is otherwise idle
                    nc.gpsimd.tensor_mul(
                        et[:, :, off : off + 128], et[:, :, off : off + 128], cm_sb
                    )
                ets_all.setdefault(jj, {})[t] = et

            # Each diag subtile qs's chain is split: a prefix over key tiles
            # 0..4j+qs-2 (emitted at group 4j+qs, needs only exp(t-2)) and a
            # 2-tile suffix + normalize at group 4j+qs+1, so at most two av
            # tiles are alive (2 small-pool slots) and the block tail is tiny.
            for t in range(tend):
                if ypq_defer:
                    ypq_defer.pop(0)()
                if t not in ets:
                    scores_group(t)
                # front-load the first 4 units (this block's v-projections
                # must land a few groups before their key tiles are consumed);
                # drain all units 2 groups before block end so the next
                # block's qk projections (and their k2x hop) are ready
                uend = max(
                    min(t + 1, 4, nunits),
                    min(nunits, (t + 1) * nunits // max(1, tend - 2)),
                )
                while upos < uend:
                    units[upos]()
                    upos += 1
                d = t - 4 * j if causal else -1
                if causal and 0 <= d:
                    if 4 * j + d - 2 >= 0:
                        av_chain(j, d, ets, 0, 4 * j + d - 2, avst)
                    if d >= 1:
                        qs = d - 1
                        av_chain(j, qs, ets, max(0, 4 * j + qs - 1), 4 * j + qs, avst)
                        finish_qs(j, qs, avst)
            if causal:
                while ypq_defer:
                    ypq_defer.pop(0)()
                av_chain(j, 3, ets, max(0, 4 * j + 2), 4 * j + 3, avst)
                if prefetch_next and j < NJ - 1:
                    scores_group(0, jj=j + 1)
                finish_qs(j, 3, avst, on_act=(j < NJ - 1))
                while ypq_defer:
                    ypq_defer.pop(0)()
            else:
                for qs in range(4):
                    av_chain(j, qs, ets, 0, NT - 1, avst)
                    finish_qs(j, qs, avst)
            while upos < nunits:
                units[upos]()
                upos += 1

        if causal:
            # Filler placement balances PE work against each block's Act
            # (exp) load, which grows ~linearly with j: v-projections are
            # deferred into their OWN block (legal: tile 4j+st is first read
            # two groups later), yproj of finished blocks goes to the late,
            # Act-heavy blocks.
            for u in [vp_unit(t) for t in range(4)] + [
                u for m in (2, 0, 1) for u in qk_units(0, m)
            ]:
                u()
            ys = [{} for _ in range(NJ)]
            # qk(j+1) leads each unit list: the next block's first scores
            # gate on it, and the front-load rule pops one unit per group
            # m2 first within each qk triple: it triggers the k2x re-basing
            # DMA whose latency otherwise stalls the next block's first scores
            units_by_block = {
                0: [u for m in (2, 0, 1) for u in qk_units(1, m)],
                1: qk_units(2, 2)
                + [vp_unit(4 + st) for st in range(4)]
                + qk_units(2, 0) + qk_units(2, 1),
                2: qk_units(3, 2)
                + [vp_unit(8 + st) for st in range(4)]
                + qk_units(3, 0) + qk_units(3, 1)
                + [u for dt in range(KC) for u in yp_units(0, dt, ys[0])],
                3: [vp_unit(12 + st) for st in range(4)]
                + [u for dt in range(KC) for u in yp_units(1, dt, ys[1])]
                + [u for dt in range(KC) for u in yp_units(2, dt, ys[2])],
            }
            for j in range(NJ):
                attend_block(j, units_by_block[j])
            # (last block's output projection is emitted per q-subtile inside
            # finish_qs so it pipelines into the attention tail)
        else:
            for t in range(NT):
                vp_unit(t)()
            for j in range(NJ):
                for m in range(3):
                    qk_unit(j, m)()
            for j in range(NJ):
                attend_block(j, [])
                ystate = {}
                for dt in range(KC):
                    for u in yp_units(j, dt, ystate):
                        u()

    nc.finalize()
    return nc


_NC_CACHE: dict[bool, object] = {}


def get_nc(causal: bool):
    if causal not in _NC_CACHE:
        _NC_CACHE[causal] = build_nc(causal)
    return _NC_CACHE[causal]


def make_in_maps(x, wq, bq, wk, bk, wv, bv, wo, bo):
    """Shard full inputs into 8 per-core input maps (all-fp16 operands)."""
    f16 = np.float16
    tri = (np.arange(128)[:, None] <= np.arange(128)[None, :]).astype(f16)
    cm = np.ascontiguousarray(np.tile(tri, (1, HG)))
    ident = np.ascontiguousarray(np.eye(128, dtype=f16))
    x = np.asarray(x, np.float32)
    in_maps = []
    for core in range(NCORES):
        b, hg = divmod(core, NH // HG)
        hs = slice(hg * HD, (hg + 1) * HD)
        qc, kc, vc = (np.asarray(w, np.float32)[hs, :].T for w in (wq, wk, wv))
        wqkT = np.concatenate(
            [qc[:, 0:128], kc[:, 0:128], qc[:, 128:HD], kc[:, 128:HD]], axis=1
        ).astype(f16)
        # pre-rearrange to partition-major [128, KC*cols] so each DMA line is
        # one contiguous run
        wqkP = wqkT.reshape(KC, 128, 2 * HD).transpose(1, 0, 2).reshape(128, -1)
        wvP = vc.astype(f16).reshape(KC, 128, HD).transpose(1, 0, 2).reshape(128, -1)
        bqh = np.asarray(bq, np.float32)[hs]
        bq1p = np.concatenate([bqh[128:HD], np.zeros(64, np.float32)])
        in_maps.append(
            {
                "xT": np.ascontiguousarray(
                    x[b].T.reshape(KC, 128, NT, 128).transpose(1, 2, 0, 3)
                ).astype(f16).reshape(128, -1),
                "wqkP": np.ascontiguousarray(wqkP),
                "wvP": np.ascontiguousarray(wvP),
                "woT": np.ascontiguousarray(np.asarray(wo, np.float32)[:, hs].T).astype(f16),
                "bq0": np.ascontiguousarray(bqh[0:128]),
                "bq1": np.ascontiguousarray(bq1p),
                "cmask": cm,
                "ident": ident,
            }
        )
    return in_maps


def combine_outputs(results, wo, bv, bo):
    """Sum head-group partials per batch, transpose, add bv@wo.T + bo."""
    y = np.empty((B, S, D), np.float32)
    ng = NH // HG
    corr = (
        np.asarray(bv, np.float64) @ np.asarray(wo, np.float64).T
        + np.asarray(bo, np.float64)
    ).astype(np.float32)
    for b in range(B):
        acc = None
        for g in range(ng):
            r = results[b * ng + g]
            yT = r["yT"].astype(np.float32)
            # block-3 columns arrive tile-major in yTq: [128, qs, dt, w]
            yTq = r["yTq"].astype(np.float32).reshape(128, 4, KC, 128)
            yT[:, 3 * SB :] = yTq.transpose(2, 0, 1, 3).reshape(D, SB)
            acc = yT if acc is None else acc + yT
        y[b] = acc.T + corr[None, :]
    return y


def kernel(x, wq, bq, wk, bk, wv, bv, wo, bo, mask, _trace=False):
    from concourse.bass_utils import run_bass_kernel_spmd

    causal = bool(np.asarray(mask).item())
    nc = get_nc(causal)
    in_maps = make_in_maps(x, wq, bq, wk, bk, wv, bv, wo, bo)
    res = run_bass_kernel_spmd(nc, in_maps, list(range(NCORES)), trace=_trace)
    y = combine_outputs(res.results, wo, bv, bo)
    if _trace:
        return y, res
    return y



# revision 7
# speedup vs baseline: 1.0159x; 1.0009x over previous
"""Trainium2 Bass kernel for 12-head causal MHA (B=2, S=2048, D=768), fp32 in/out.

Sharding: 8 cores = (batch b in {0,1}) x (head-group hg in {0..3}, 3 heads each).
All matmul operands are fp16 (1 cycle/row on PE, half the DMA bytes); PSUM
accumulation is fp32, so end-to-end rel err is ~7e-4 (budget 2e-2).

Math notes (exact, not approximations):
  - bk drops out of softmax entirely: (q+bq)·(k+bk) = q·k + bq·k + const(row).
  - bv folds to the end: attn rows sum to 1, so attn@(v+bv) = attn@v + bv;
    host adds bv@wo.T + bo once.
  - Only bq is applied on device (per-partition scalar add on the q rows).

Per-core layout/dataflow (block j = 512 q columns, key tile t = 128 keys):
  qkT  [384, S]   packed ([q0;q1] | [k0;k1] | [q2;k2]) in 3 m-groups so each
                  head's (q,k) share a base partition; k2 is re-based to
                  partitions 0..63 by a small SBUF->SBUF DMA (Pool queue).
  xT   [128, NT, KC, 128] tile-major (host pre-arranged): every xT DMA line
                  is one contiguous >=1.5KB run, and the first key tile
                  arrives ~0.6us earlier at kernel start.
  v65  [keys, 65] per (tile, head): v columns + a ones column (softmax denom).
  scores: psum [128 keys, 3 heads, <=512 q] (3 banks), one matmul per head,
          causally trimmed; ONE exp for all 3 heads (Act engine), then the
          diagonal 128-col window is masked in place (Pool, SBUF-only op).
  attnV: out [128 q-subtile, 3*65] in ONE psum bank, chains accumulated over
          persistent et tiles (rhs=v65 moving, ap=65 -> 4x fewer PE cycles
          than the [65, q] layout); chain split into prefix + 2-tile suffix
          so finish work overlaps the Act-paced diagonal groups.
  normalize: denominator is a COLUMN -> per-partition reciprocal + scalar mul
          (no partition broadcast needed).
  transpose: blocks 0-2 use one xbar DMA transpose [q,256]->outT (frees PE
          cycles + both DVE copies); the last block keeps the low-latency
          PE transpose since ypq consumes outT immediately.
  yproj: yT[768, S] partial per head-group, host sums 4 partials per batch;
          the last block's yproj runs per q-subtile inside the tail.

Engines: PE matmuls; Act exp (the attention pacer) + tail copies; DVE psum
copies/bias/normalize; Pool masks + k2x DMAs (GPSIMD cannot access PSUM).
PSUM: scores get a dedicated 2x3-bank slot set, everything else (<=2KB)
shares a 2x1-bank pool; start=True lazily zeroes a whole 2KB bank, so
multi-chain banks start exactly once and accumulate onto pending-zero bytes.
Pipeline: projection chains of later blocks and yproj of earlier blocks are
interleaved into attend(j)'s key-tile groups, weighted so each block stays
PE-bound against its linearly-growing exp load.
"""

import math
from contextlib import ExitStack

import numpy as np

import concourse.bacc as bacc
import concourse.bass as bass
import concourse.mybir as mybir
import concourse.tile as tile

FP32 = mybir.dt.float32
FP16 = mybir.dt.float16

B = 2
S = 2048
D = 768
NH = 12
DK = 64
NCORES = 8
HG = 3  # heads per core
HD = HG * DK  # 192
KC = D // 128  # 6 contraction chunks
SB = 512  # q block
NJ = S // SB  # 4
NT = S // 128  # 16 key tiles
SCALE = 1.0 / math.sqrt(DK)
EXP = mybir.ActivationFunctionType.Exp

# head slices inside the packed qkT layout. Matmul operands must share a base
# partition, so groups are m0=[q0;q1], m1=[k0;k1], m2=[q2;k2]; k2 is DMA-moved
# to partitions 0..63 of a scratch tile so h2's (q2,k2) pair is base-aligned.
_Q_SL = {0: (0, 0), 1: (0, 64), 2: (2, 0)}
_K_SL = {0: (1, 0), 1: (1, 64)}


def build_nc(causal: bool):
    nc = bacc.Bacc(trn_type="TRN2", target_bir_lowering=False, debug=False)

    xT_d = nc.declare_dram_parameter("xT", [128, NT * KC * 128], FP16, isOutput=False)
    # weights arrive pre-rearranged to partition-major [128, KC*cols] on the
    # host so each DMA line is one long contiguous run (full DMA speed)
    wqk_d = nc.declare_dram_parameter("wqkP", [128, KC * 2 * HD], FP16, isOutput=False)
    wv_d = nc.declare_dram_parameter("wvP", [128, KC * HD], FP16, isOutput=False)
    woT_d = nc.declare_dram_parameter("woT", [HD, D], FP16, isOutput=False)
    bq0_d = nc.declare_dram_parameter("bq0", [128], FP32, isOutput=False)
    bq1_d = nc.declare_dram_parameter("bq1", [128], FP32, isOutput=False)
    cm_d = nc.declare_dram_parameter("cmask", [128, HG * 128], FP16, isOutput=False)
    id_d = nc.declare_dram_parameter("ident", [128, 128], FP16, isOutput=False)
    yT_d = nc.declare_dram_parameter("yT", [D, S], FP16, isOutput=True)
    # block-3 columns land tile-major here: contiguous 768B runs per
    # partition-half -> mult-1 DMA speed in the latency-critical kernel tail
    yTq_d = nc.declare_dram_parameter("yTq", [128, 4 * KC * 128], FP16, isOutput=True)

    with tile.TileContext(nc) as tc, ExitStack() as ctx:
        consts = ctx.enter_context(tc.tile_pool(name="consts", bufs=1))

        xT_sb = consts.tile([128, NT, KC, 128], FP16)
        wqk_sb = consts.tile([128, KC, 2 * HD], FP16)  # packed q|k m-groups
        wv_sb = consts.tile([128, KC, HD], FP16)
        woT0_sb = consts.tile([128, D], FP16)
        woT1_sb = consts.tile([64, D], FP16)
        bq0_sb = consts.tile([128, 1], FP32)
        bq1_sb = consts.tile([128, 1], FP32)
        cm_sb = consts.tile([128, HG, 128], FP16)  # triangle keep-mask x3 heads
        id_sb = consts.tile([128, 128], FP16)
        qkT_sb = consts.tile([128, 3, S], FP16)
        k2x_sb = consts.tile([64, S], FP16)  # k2 re-based to partitions 0..63
        v65_sb = consts.tile([128, NT, HG, 65], FP16)
        # outT[:, 0, s] = head-dims 0..127; outT[0:64, 1, s] = dims 128..191
        # (rows 64:128 of half 1 are junk from the padded transpose)
        outT_sb = consts.tile([128, 2, S], FP16)

        # ---- input DMAs, all on the SP queue. Order matters: v-weights and
        # x block 0 land first so the first vproj chain starts ~2us in.
        xT_r = xT_d.ap().rearrange("p (t c w) -> p t c w", t=NT, c=KC)

        def load_x_block(jb):
            nc.sync.dma_start(
                out=xT_sb[:, 4 * jb : 4 * jb + 4],
                in_=xT_r[:, 4 * jb : 4 * jb + 4],
            )

        # first transfers fan out across SP/DVE/Act queues so their fixed
        # issue latencies (seq+hwdge+dge ~1.8us each) overlap at kernel start
        nc.sync.dma_start(
            out=wv_sb, in_=wv_d.ap().rearrange("p (c n) -> p c n", c=KC)
        )
        nc.gpsimd.dma_start(out=xT_sb[:, 0:1], in_=xT_r[:, 0:1])
        nc.scalar.dma_start(out=xT_sb[:, 1:4], in_=xT_r[:, 1:4])
        nc.sync.dma_start(
            out=wqk_sb, in_=wqk_d.ap().rearrange("p (c n) -> p c n", c=KC)
        )
        nc.sync.dma_start(out=bq0_sb, in_=bq0_d.ap().rearrange("(p o) -> p o", o=1))
        nc.sync.dma_start(out=bq1_sb, in_=bq1_d.ap().rearrange("(p o) -> p o", o=1))
        load_x_block(1)
        nc.sync.dma_start(out=cm_sb, in_=cm_d.ap().rearrange("p (h c) -> p h c", h=HG))
        nc.sync.dma_start(out=id_sb, in_=id_d.ap())
        nc.sync.dma_start(out=woT0_sb, in_=woT_d.ap()[0:128, :])
        nc.sync.dma_start(out=woT1_sb, in_=woT_d.ap()[128:HD, :])
        load_x_block(2)
        load_x_block(3)

        nc.vector.memset(v65_sb[:, :, :, 64:65], 1.0)  # softmax-denominator col
        # ---- PSUM pools: scores get a dedicated 2x3-bank slot set so the
        # next scores group never waits on exp draining a filler's slot; all
        # other psum tiles are <=2KB and share a 2x1-bank small pool.
        sc_pool = ctx.enter_context(tc.tile_pool(name="sc", bufs=2, space="PSUM"))
        fl_pool = ctx.enter_context(tc.tile_pool(name="fl", bufs=2, space="PSUM"))
        et_pool = ctx.enter_context(tc.tile_pool(name="et", bufs=18))
        nrm_pool = ctx.enter_context(tc.tile_pool(name="nrm", bufs=3))
        rc_pool = ctx.enter_context(tc.tile_pool(name="rc", bufs=3))
        yt_pool = ctx.enter_context(tc.tile_pool(name="yt", bufs=3))

        def vp_unit(t):
            """Project v for key tile t into v65 (natural layout, ap=192)."""

            def emit():
                vpu = fl_pool.tile([128, 256], FP32, name="vpu", tag="f")
                for c in range(KC):
                    nc.tensor.matmul(
                        vpu[:, 0:HD],
                        lhsT=xT_sb[:, t, c, :],
                        rhs=wv_sb[:, c, :],
                        start=(c == 0),
                        stop=(c == KC - 1),
                    )
                # one strided copy; GPSIMD cannot touch PSUM, so DVE
                nc.vector.tensor_copy(
                    v65_sb[:, t, :, 0:64],
                    vpu[:, 0:HD].rearrange("p (h d) -> p h d", h=HG),
                )

            return emit

        def qk_units(j, m):
            """Project packed qk m-group for q block j (transposed, ap=512),
            split into two filler units of 3 contraction chunks each so the
            scheduler can pace them at sub-group granularity."""
            state = {}

            def piece(c0, c1):
                def emit():
                    if c0 == 0:
                        state["qku"] = fl_pool.tile(
                            [128, SB], FP32, name="qku", tag="f"
                        )
                    for c in range(c0, c1):
                        nc.tensor.matmul(
                            state["qku"],
                            lhsT=wqk_sb[:, c, m * 128 : (m + 1) * 128],
                            rhs=xT_sb[:, 4 * j : 4 * j + 4, c, :],
                            start=(c == 0),
                            stop=(c == KC - 1),
                        )
                return emit

            emit_a = piece(0, 2)
            emit_mid = piece(2, 4)

            def emit_b():
                qku = state["qku"]
                for c in range(4, KC):
                    nc.tensor.matmul(
                        qku,
                        lhsT=wqk_sb[:, c, m * 128 : (m + 1) * 128],
                        rhs=xT_sb[:, 4 * j : 4 * j + 4, c, :],
                        start=False,
                        stop=(c == KC - 1),
                    )
                jsl = slice(j * SB, (j + 1) * SB)
                dst = qkT_sb[:, m, jsl]
                if m == 0:
                    nc.vector.tensor_scalar_add(dst, qku, bq0_sb)
                elif m == 2:
                    nc.vector.tensor_scalar_add(dst, qku, bq1_sb)
                    # Pool (swdge) queue: the SP queue is busy streaming bulk
                    # inputs and would delay this small hop by several us
                    nc.gpsimd.dma_start(
                        out=k2x_sb[0:64, jsl], in_=qkT_sb[64:128, 2, jsl]
                    )
                else:
                    nc.vector.tensor_copy(dst, qku)

            return [emit_a, emit_mid, emit_b]

        def yp_units(j, dt, ystate):
            """Output projection for one 128-row chunk of yT, q block j."""

            def emit():
                ypu = fl_pool.tile([128, SB], FP32, name="ypu", tag="f")
                jsl = slice(j * SB, (j + 1) * SB)
                nc.tensor.matmul(
                    ypu,
                    lhsT=woT0_sb[:, dt * 128 : (dt + 1) * 128],
                    rhs=outT_sb[:, 0, jsl],
                    start=True,
                    stop=False,
                )
                nc.tensor.matmul(
                    ypu,
                    lhsT=woT1_sb[0:64, dt * 128 : (dt + 1) * 128],
                    rhs=outT_sb[0:64, 1, jsl],
                    start=False,
                    stop=True,
                )
                sub = dt % 3
                if sub == 0:
                    ystate["yt"] = yt_pool.tile([128, 3, SB], FP16, name="yt")
                nc.vector.tensor_copy(ystate["yt"][:, sub, :], ypu)
                if sub == 2:
                    nc.sync.dma_start(
                        out=yT_d.ap().rearrange("(g p) s -> p g s", p=128)[
                            :, dt - 2 : dt + 1, jsl
                        ],
                        in_=ystate["yt"],
                    )

            return [emit]

        def av_chain(j, qs, ets, t_lo, t_hi, avst):
            """attnV accumulation for q-subtile qs of block j over key tiles
            t_lo..t_hi. av psum unit is [128, 3*65] in ONE bank: only the
            first write may use start=True (it lazily zeroes the whole bank);
            later chains accumulate onto pending-zero bytes."""
            if t_lo == 0:
                avst[qs] = fl_pool.tile([128, 256], FP32, name="av", tag="f")
            av = avst[qs]
            for t in range(t_lo, t_hi + 1):
                for h in range(HG):
                    nc.tensor.matmul(
                        av[:, 65 * h : 65 * h + 65],
                        lhsT=ets[t][:, h, qs * 128 : (qs + 1) * 128],
                        rhs=v65_sb[:, t, h, :],
                        start=(t == 0 and h == 0),
                        stop=(t == t_hi),
                        skip_group_check=True,
                    )

        ypq_defer = []

        def finish_qs(j, qs, avst, on_act=False):
            """Normalize q-subtile qs of block j and transpose into outT.

            on_act is unused (kept for experiment parity)."""
            av = avst[qs]
            rc = rc_pool.tile([128, HG], FP32, name="rc")
            nc.vector.reciprocal(rc, av[:, 64 : 65 * HG : 65])
            nrm = nrm_pool.tile([128, 256], FP16, name="nrm")
            nc.gpsimd.memset(nrm[:, HD:256], 0.0)
            for h in range(HG):
                nc.vector.tensor_scalar_mul(
                    nrm[:, h * 64 : (h + 1) * 64],
                    av[:, 65 * h : 65 * h + 64],
                    rc[:, h : h + 1],
                )
            col = slice(j * SB + qs * 128, j * SB + (qs + 1) * 128)
            if j < NJ - 1:
                # single xbar DMA transpose (SP queue): [128, 256] -> both
                # outT halves at once. Frees the PE transposes and both DVE
                # psum->sbuf copies; outT is consumed 1-2 blocks later so the
                # dge latency is invisible.
                nc.sync.dma_start_transpose(outT_sb[:, :, col], nrm)
            else:
                # last block: outT feeds ypq immediately; keep the low-latency
                # PE transpose. both land in one psum bank: the second must
                # not start (start lazily zeroes the bank, wiping the first)
                tp = fl_pool.tile([128, 256], FP16, name="tp", tag="f")
                nc.tensor.transpose(tp[0:128, 0:128], nrm[:, 0:128], id_sb)
                nc.tensor.matmul(
                    tp[0:64, 128:256],
                    lhsT=nrm[:, 128:HD],
                    rhs=id_sb,
                    is_transpose=True,
                    start=False,
                    stop=True,
                    skip_group_check=True,
                )
                nc.vector.tensor_copy(outT_sb[:, 0, col], tp[0:128, 0:128])
                nc.vector.tensor_copy(outT_sb[0:64, 1, col], tp[0:64, 128:256])
            if causal and j == NJ - 1:
                # last block: project this q-subtile's columns; emission is
                # deferred one group so the PE matmuls overlap the NEXT
                # subtile's DVE normalize/copy phase instead of waiting on
                # this one's.
                def ypq_emit(qs=qs, col=col):
                    _ypq_body(j, qs, col)
                ypq_defer.append(ypq_emit)

        def _ypq_body(j, qs, col):
            if True:
                ytq = yt_pool.tile([128, KC, 128], FP16, name="ytq", tag="ytq")
                for half in range(2):
                    ypq = fl_pool.tile([128, 3, 128], FP32, name="ypq", tag="f")
                    for sub in range(3):
                        dt = 3 * half + sub
                        nc.tensor.matmul(
                            ypq[:, sub, :],
                            lhsT=woT0_sb[:, dt * 128 : (dt + 1) * 128],
                            rhs=outT_sb[:, 0, col],
                            start=(sub == 0),
                            stop=False,
                            skip_group_check=True,
                        )
                        nc.tensor.matmul(
                            ypq[:, sub, :],
                            lhsT=woT1_sb[0:64, dt * 128 : (dt + 1) * 128],
                            rhs=outT_sb[0:64, 1, col],
                            start=False,
                            stop=(sub == 2),
                            skip_group_check=True,
                        )
                    # Act: it is idle in the kernel tail where these run,
                    # while DVE is the serial bottleneck of the final rounds.
                    # For the very last q-subtile, the second half goes via
                    # DVE so the two final copy->DMA chains run in parallel.
                    if half == 1 and qs == 3:
                        nc.vector.tensor_copy(ytq[:, 3:6, :], ypq)
                    else:
                        nc.scalar.copy(ytq[:, 3 * half : 3 * half + 3, :], ypq)
                # one tile-major DMA per q-subtile (1.5KB contiguous runs,
                # mult-1): two half-DMAs would each hit the 500ns descriptor
                # floor and serialize on the shared DMA device
                nc.sync.dma_start(
                    out=yTq_d.ap().rearrange("p (q c w) -> p q c w", q=4, c=KC)[
                        :, qs
                    ],
                    in_=ytq,
                )

        ets_all = {}

        def attend_block(j, units, prefetch_next=True):
            """Attention for q block j; pops filler units between key tiles.

            attnV chains run per q-subtile over persistent et tiles, one
            group after the diagonal tile's exp, so the PE never waits on
            the Act engine in program order. The next block's first scores
            group is emitted inside this block's last iteration: its psum
            half is free by then, and its exp fills Act's boundary bubble.
            """
            tend = 4 * (j + 1) if causal else NT
            ets = ets_all.setdefault(j, {})
            del ypq_defer[:]
            avst = {}
            nunits = len(units)
            upos = 0

            def scores_group(t, jj=j):
                d = t - 4 * jj if causal else -1  # diag index within block
                off = 128 * d if (causal and d >= 0) else 0
                sp = sc_pool.tile([128, HG, SB], FP32, name="sp", tag="s")
                for h in range(HG):
                    if h < 2:
                        kg, kp = _K_SL[h]
                        k_ap = qkT_sb[kp : kp + 64, kg, t * 128 : (t + 1) * 128]
                    else:
                        k_ap = k2x_sb[0:64, t * 128 : (t + 1) * 128]
                    qg, qp = _Q_SL[h]
                    nc.tensor.matmul(
                        sp[:, h, off:],
                        lhsT=k_ap,
                        rhs=qkT_sb[qp : qp + 64, qg, jj * SB + off : (jj + 1) * SB],
                        start=True,
                        stop=True,
                    )
                et = et_pool.tile([128, HG, SB], FP16, name="et")
                nc.scalar.activation(et[:, :, off:], sp[:, :, off:], EXP, scale=SCALE)
                if causal and d >= 0:
                    # intra-tile causal triangle: keep key p iff p <= local q.
                    # Pool engine: SBUF-only op, and Pool is otherwise idle
                    nc.gpsimd.tensor_mul(
                        et[:, :, off : off + 128], et[:, :, off : off + 128], cm_sb
                    )
                ets_all.setdefault(jj, {})[t] = et

            # Each diag subtile qs's chain is split: a prefix over key tiles
            # 0..4j+qs-2 (emitted at group 4j+qs, needs only exp(t-2)) and a
            # 2-tile suffix + normalize at group 4j+qs+1, so at most two av
            # tiles are alive (2 small-pool slots) and the block tail is tiny.
            for t in range(tend):
                if ypq_defer:
                    ypq_defer.pop(0)()
                if t not in ets:
                    scores_group(t)
                # front-load the first 4 units (this block's v-projections
                # must land a few groups before their key tiles are consumed);
                # drain all units 2 groups before block end so the next
                # block's qk projections (and their k2x hop) are ready
                uend = max(
                    min(t + 1, 4, nunits),
                    min(nunits, (t + 1) * nunits // max(1, tend - 2)),
                )
                while upos < uend:
                    units[upos]()
                    upos += 1
                d = t - 4 * j if causal else -1
                if causal and 0 <= d:
                    if 4 * j + d - 2 >= 0:
                        av_chain(j, d, ets, 0, 4 * j + d - 2, avst)
                    if d >= 1:
                        qs = d - 1
                        av_chain(j, qs, ets, max(0, 4 * j + qs - 1), 4 * j + qs, avst)
                        finish_qs(j, qs, avst)
            if causal:
                while ypq_defer:
                    ypq_defer.pop(0)()
                av_chain(j, 3, ets, max(0, 4 * j + 2), 4 * j + 3, avst)
                if prefetch_next and j < NJ - 1:
                    scores_group(0, jj=j + 1)
                finish_qs(j, 3, avst, on_act=(j < NJ - 1))
                while ypq_defer:
                    ypq_defer.pop(0)()
            else:
                for qs in range(4):
                    av_chain(j, qs, ets, 0, NT - 1, avst)
                    finish_qs(j, qs, avst)
            while upos < nunits:
                units[upos]()
                upos += 1

        if causal:
            # Filler placement balances PE work against each block's Act
            # (exp) load, which grows ~linearly with j: v-projections are
            # deferred into their OWN block (legal: tile 4j+st is first read
            # two groups later), yproj of finished blocks goes to the late,
            # Act-heavy blocks.
            for u in [vp_unit(t) for t in range(4)] + [
                u for m in (2, 0, 1) for u in qk_units(0, m)
            ]:
                u()
            ys = [{} for _ in range(NJ)]
            # qk(j+1) leads each unit list: the next block's first scores
            # gate on it, and the front-load rule pops one unit per group
            # m2 first within each qk triple: it triggers the k2x re-basing
            # DMA whose latency otherwise stalls the next block's first scores
            units_by_block = {
                0: [u for m in (2, 0, 1) for u in qk_units(1, m)],
                1: qk_units(2, 2)
                + [vp_unit(4 + st) for st in range(4)]
                + qk_units(2, 0) + qk_units(2, 1),
                2: qk_units(3, 2)
                + [vp_unit(8 + st) for st in range(4)]
                + qk_units(3, 0) + qk_units(3, 1)
                + [u for dt in range(KC) for u in yp_units(0, dt, ys[0])],
                3: [vp_unit(12 + st) for st in range(4)]
                + [u for dt in range(KC) for u in yp_units(1, dt, ys[1])]
                + [u for dt in range(KC) for u in yp_units(2, dt, ys[2])],
            }
            for j in range(NJ):
                attend_block(j, units_by_block[j])
            # (last block's output projection is emitted per q-subtile inside
            # finish_qs so it pipelines into the attention tail)
        else:
            for t in range(NT):
                vp_unit(t)()
            for j in range(NJ):
                for m in range(3):
                    qk_unit(j, m)()
            for j in range(NJ):
                attend_block(j, [])
                ystate = {}
                for dt in range(KC):
                    for u in yp_units(j, dt, ystate):
                        u()

    nc.finalize()
    return nc


_NC_CACHE: dict[bool, object] = {}


def get_nc(causal: bool):
    if causal not in _NC_CACHE:
        _NC_CACHE[causal] = build_nc(causal)
    return _NC_CACHE[causal]


def make_in_maps(x, wq, bq, wk, bk, wv, bv, wo, bo):
    """Shard full inputs into 8 per-core input maps (all-fp16 operands)."""
    f16 = np.float16
    tri = (np.arange(128)[:, None] <= np.arange(128)[None, :]).astype(f16)
    cm = np.ascontiguousarray(np.tile(tri, (1, HG)))
    ident = np.ascontiguousarray(np.eye(128, dtype=f16))
    x = np.asarray(x, np.float32)
    in_maps = []
    for core in range(NCORES):
        b, hg = divmod(core, NH // HG)
        hs = slice(hg * HD, (hg + 1) * HD)
        qc, kc, vc = (np.asarray(w, np.float32)[hs, :].T for w in (wq, wk, wv))
        wqkT = np.concatenate(
            [qc[:, 0:128], kc[:, 0:128], qc[:, 128:HD], kc[:, 128:HD]], axis=1
        ).astype(f16)
        # pre-rearrange to partition-major [128, KC*cols] so each DMA line is
        # one contiguous run
        wqkP = wqkT.reshape(KC, 128, 2 * HD).transpose(1, 0, 2).reshape(128, -1)
        wvP = vc.astype(f16).reshape(KC, 128, HD).transpose(1, 0, 2).reshape(128, -1)
        bqh = np.asarray(bq, np.float32)[hs]
        bq1p = np.concatenate([bqh[128:HD], np.zeros(64, np.float32)])
        in_maps.append(
            {
                "xT": np.ascontiguousarray(
                    x[b].T.reshape(KC, 128, NT, 128).transpose(1, 2, 0, 3)
                ).astype(f16).reshape(128, -1),
                "wqkP": np.ascontiguousarray(wqkP),
                "wvP": np.ascontiguousarray(wvP),
                "woT": np.ascontiguousarray(np.asarray(wo, np.float32)[:, hs].T).astype(f16),
                "bq0": np.ascontiguousarray(bqh[0:128]),
                "bq1": np.ascontiguousarray(bq1p),
                "cmask": cm,
                "ident": ident,
            }
        )
    return in_maps


def combine_outputs(results, wo, bv, bo):
    """Sum head-group partials per batch, transpose, add bv@wo.T + bo."""
    y = np.empty((B, S, D), np.float32)
    ng = NH // HG
    corr = (
        np.asarray(bv, np.float64) @ np.asarray(wo, np.float64).T
        + np.asarray(bo, np.float64)
    ).astype(np.float32)
    for b in range(B):
        acc = None
        for g in range(ng):
            r = results[b * ng + g]
            yT = r["yT"].astype(np.float32)
            # block-3 columns arrive tile-major in yTq: [128, qs, dt, w]
            yTq = r["yTq"].astype(np.float32).reshape(128, 4, KC, 128)
            yT[:, 3 * SB :] = yTq.transpose(2, 0, 1, 3).reshape(D, SB)
            acc = yT if acc is None else acc + yT
        y[b] = acc.T + corr[None, :]
    return y


def kernel(x, wq, bq, wk, bk, wv, bv, wo, bo, mask, _trace=False):
    from concourse.bass_utils import run_bass_kernel_spmd

    causal = bool(np.asarray(mask).item())
    nc = get_nc(causal)
    in_maps = make_in_maps(x, wq, bq, wk, bk, wv, bv, wo, bo)
    res = run_bass_kernel_spmd(nc, in_maps, list(range(NCORES)), trace=_trace)
    y = combine_outputs(res.results, wo, bv, bo)
    if _trace:
        return y, res
    return y

